# revision 10
# baseline (speedup 1.0000x reference)
"""DTW on 8 NeuronCores — 128-partition wavefront version.

Layout: partition p = b*8 + g  (b in [0,16) batches, g in [0,8) row-groups
of 64 rows each; n = 512 = 8*64).  The DTW column recurrence
    R[i,j] = D[i,j] + min(prev[i-1], prev[i], R[i-1,j])
is run as a wavefront: at step s, group g processes columns
j in [c*(s-g), c*(s-g)+c).  Group g's scan init / halo come from group
g-1 (partition p-1) via stream_shuffle of the carry columns.

Per column: 2 main DVE ops ([128,64] min2 + fused scan
    state = min(min2[k], state) + D[k]
) + per step: 1 shuffle + 1 fix (boundary BIG for g=0 rows), amortized
over c columns.

D delivery: Phase A matmuls (as v1) -> PSUM [128 j, 512 i] per (b, jc);
ACT copy downcasts to bf16 stripe stb[j, (b i)]; store to DRAM [b][j][i];
reload per (g, jc) into the skewed SBUF tile
    Dsk[p = b*8+g][slot t = j + g*c][k]  (bf16, 128B descriptors).
Slots outside [g*c, 512+g*c) are memset to BIG so idle wavefront steps
produce huge-but-finite values that never win a min against real paths.
"""

import numpy as np

import concourse.bass as bass
import concourse.tile as tile
from concourse import mybir
from concourse.bass_utils import run_bass_kernel_spmd

F32 = mybir.dt.float32
BF16 = mybir.dt.bfloat16
BIG = 1e30
NCORES = 8
G = 8          # row groups per batch
C = 4          # column unroll per wavefront step


def build_kernel(nb, n, m, d, c=C):
    P = 128
    assert n % P == 0 and m % P == 0 and d <= 126
    assert nb * G == P and n % G == 0 and m % c == 0
    K = d + 2
    njc = m // P
    kk = n // G                      # rows per group (64)
    o = 2 * c                        # group-to-group column offset
    fw = (G - 1) * o                 # max skew (56 for c=4)
    nslot = m + fw                   # skewed slot count (568 for c=4)
    S = m // c + 2 * (G - 1)         # wavefront steps

    nc = bass.Bass()
    in_d = nc.dram_tensor("allin", [K, nb * (n + m)], BF16,
                          kind="ExternalInput")
    cons_d = nc.dram_tensor("consts", [P, 2 + 2 * c], F32, kind="ExternalInput")
    bigd = nc.dram_tensor("bigd", [P, (G - 1) * 2 * c * kk], BF16,
                          kind="ExternalInput")
    out_d = nc.dram_tensor("out", [P, 1], F32, kind="ExternalOutput")

    import bass_rust

    with tile.TileContext(nc) as tc:
        with (
            tc.tile_pool(name="singles", bufs=1) as singles,
            tc.tile_pool(name="stage", bufs=2) as stage,
            tc.tile_pool(name="psmm", bufs=3, space="PSUM") as psmm,
            tc.tile_pool(name="dram", bufs=1, space="DRAM") as dram,
        ):
            # DRAM copy of D, already skewed: row (b*8+g), free [slot][k],
            # slot = j + g*c
            Ddram2 = dram.tile([P, nslot * kk], BF16)
            Dsk = singles.tile([P, nslot * kk], BF16)
            Pb = singles.tile([P, 2 * c * 65], F32)   # 2c prev buffers [halo|64]
            cons = singles.tile([P, 2 + 2 * c], F32)
            tmp = singles.tile([P, 2 * c], F32)
            mm = singles.tile([P, kk], F32)
            allinT = singles.tile([K, nb * (n + m)], BF16)

            wz = singles.tile([2, n], BF16)

            nc.sync.dma_start(cons[:], cons_d[:, :])
            nc.scalar.memzero(wz[:])

            # packed inputs (bf16), one resident tile [K, b*(n+m)], host
            # pre-transposed so each batch is a plain contiguous copy.
            # Round-robin over three issue queues (the cost of a DMA sits
            # on its issuing engine, so queues are parallel bandwidth).
            qs = [nc.sync, nc.scalar]
            Q = n + m
            QA = P + n          # y-stripe0 + x: stripe-0's working set
            for b in range(nb):
                qs[b % 2].dma_start(allinT[:, b * Q:b * Q + QA],
                                    in_d[:, b * Q:b * Q + QA])

            # PE warm-up so the first real matmul skips the cold pstate
            for w in range(2):
                pdw = psmm.tile([P, n], F32, tag="pd")
                nc.tensor.matmul(pdw[:], wz[0:2, 0:P], wz[0:2, 0:n],
                                 start=True, stop=True)

            # ---- prev init ----
            nc.vector.memset(Pb[:], BIG)
            # buffer c-1 halo: 0 for g==0 rows (R[-1,-1]), BIG otherwise
            nc.scalar.copy(Pb[:, (2 * c - 1) * 65:(2 * c - 1) * 65 + 1],
                           cons[:, 0:1])

            def al(b):
                return allinT[:, b * (n + m):(b + 1) * (n + m)]

            def aly(b, jc):      # y-stripe jc lhsT columns
                base = b * (n + m)
                off = 0 if jc == 0 else n + jc * P
                return allinT[:, base + off:base + off + P]

            def alx(b, lo, hi):  # x columns [lo, hi)
                base = b * (n + m) + P
                return allinT[:, base + lo:base + hi]

            # BIG prefill of the never-stored DRAM slot bands (front
            # [0, fw): slots below each row's skew; back [m, m+fw): beyond
            # each row's last column) from a host-filled constant
            ddv2 = Ddram2[:].rearrange("p (s k) -> p s k", k=kk)
            bigv = bigd[:].rearrange("p (s k) -> p s k", k=kk)
            nc.sync.dma_start(ddv2[:, 0:fw, :], bigv)
            nc.sync.dma_start(ddv2[:, m:m + fw, :], bigv)

            # skewed store views: dims (s, b, k) for fixed g
            ddv2g = Ddram2[:].rearrange("(b g) (s k) -> g b s k", g=G, k=kk)

            # ---- Phase A: D^T stripes -> skewed DRAM -> SBUF windows ----
            # Stripe 0 is on the wavefront's critical path.  Slot sigma
            # only needs rows with g*c <= sigma (higher groups are still in
            # their BIG prefix), so the first load [0,16) needs only
            # g<=3 = rows i<256: compute stripe 0 in i-halves and store/
            # load in j-chunks.  Stores go on ACT/Pool queues; loads on SP.
            # Pool runs the wavefront, so Phase A stays off it except
            # stripe-0 copies (which precede the wavefront in Pool order)
            queues = [nc.scalar, nc.sync]

            def copy_pd(stb, pd, b, ilo, ihi, allow_dve):
                dst = stb[:, b * n + ilo:b * n + ihi]
                if allow_dve and b % 2:
                    nc.vector.tensor_copy(dst, pd[:, 0:ihi - ilo])
                else:
                    nc.scalar.copy(dst, pd[:, 0:ihi - ilo])

            def store(stbv, jc, g, ja, jb, q=None):
                lo = jc * P + g * o
                (queues[g % 2] if q is None else queues[q]).dma_start(
                    ddv2g[g][:, lo + ja:lo + jb, :]
                    .rearrange("b s k -> s b k"),
                    stbv[g][ja:jb])

            def load(lo2, hi2):
                # lagged rectangular reload: slot sigma is complete for
                # every row g once columns up to sigma are stored (front
                # band is BIG)
                nc.sync.dma_start(Dsk[:, lo2 * kk:hi2 * kk],
                                  Ddram2[:, lo2 * kk:hi2 * kk])

            stb = stage.tile([P, nb * n], BF16, tag="stb")
            stbv = stb[:].rearrange("j (b g k) -> g j b k", g=G, k=kk)
            # i-half 0 (g 0..3) first: full-width g0-3 stores serve both
            # the first load [0,16) (g4-7 rows there are still BIG front
            # band, no dependency) and the later windows
            for b in range(nb):
                pd = psmm.tile([P, n // 2], F32, tag="pd")
                nc.tensor.matmul(
                    pd[:], aly(b, 0), alx(b, 0, n // 2),
                    start=True, stop=True)
                dsth0 = stb[:, b * n:b * n + n // 2]
                if b % 4 != 0:
                    nc.vector.tensor_copy(dsth0, pd[:, 0:n // 2])
                else:
                    nc.scalar.copy(dsth0, pd[:, 0:n // 2])
            for g in range(G // 2):
                store(stbv, 0, g, 0, P, q=g % 2)
            load(0, 32)
            # i-half 1 (g 4..7)
            for b in range(nb):
                pd = psmm.tile([P, n // 2], F32, tag="pd")
                nc.tensor.matmul(
                    pd[:], aly(b, 0), alx(b, n // 2, n),
                    start=True, stop=True)
                dsth1 = stb[:, b * n + n // 2:(b + 1) * n]
                if b % 4 != 0:
                    nc.vector.tensor_copy(dsth1, pd[:, 0:n // 2])
                else:
                    nc.scalar.copy(dsth1, pd[:, 0:n // 2])
            for g in range(G // 2, G):
                store(stbv, 0, g, 0, P, q=g % 2)
            load(32, 64)
            load(64, P)

            for b in range(nb):
                qs[b % 2].dma_start(allinT[:, b * Q + QA:(b + 1) * Q],
                                    in_d[:, b * Q + QA:(b + 1) * Q])

            # ---- stripes 1..3: upfront emission; copies on ACT only
            # (Pool runs the wavefront; engine streams are in-order)
            for jc in range(1, njc):
                st = stage.tile([P, nb * n], BF16, tag="stb")
                sv = st[:].rearrange("j (b g k) -> g j b k", g=G, k=kk)
                for b in range(nb):
                    pd = psmm.tile([P, n], F32, tag="pd")
                    nc.tensor.matmul(
                        pd[:], aly(b, jc), alx(b, 0, n),
                        start=True, stop=True)
                    nc.scalar.copy(st[:, b * n:(b + 1) * n], pd[:])
                for i in range(G):
                    store(sv, jc, i, 0, P, q=i % 2)
                load(jc * P, jc * P + P // 2)
                load(jc * P + P // 2, (jc + 1) * P)
            # back band: BIG-prefilled, real tails from last stores
            load(m, nslot)

            # ---- Phase B: wavefront ----
            # min2 + scans on Pool (cheap tensor ops); the cross-partition
            # carry shuffle + g0-boundary fix on DVE.  min2 for q=0 of step
            # s+1 is hoisted right after scan c-1 of step s so the DVE fix
            # (which overwrites the halos min2_0 reads) never stalls Pool.
            pbv = Pb[:].rearrange("p (q e) -> p q e", e=65)
            shuffle_mask = [0] + list(range(31))
            # prologue: min2 for step 0 col 0 (reads BIG-init buffer 2c-1)
            nc.vector.tensor_tensor(
                mm[:], pbv[:, 2 * c - 1, 0:64], pbv[:, 2 * c - 1, 1:65],
                mybir.AluOpType.min)
            for s in range(S):
                for q in range(c):
                    B = (s % 2) * c + q          # this column's buffer
                    if q == 0 and s % 2 == 0:
                        # carries of g-1's last-2-step columns -> halos
                        nc.vector.stream_shuffle(
                            tmp[:], pbv[:, :, 64], shuffle_mask)
                        nc.vector.scalar_tensor_tensor(
                            pbv[:, :, 0], tmp[:], cons[:, 1:2],
                            cons[:, 2:2 + 2 * c],
                            mybir.AluOpType.mult, mybir.AluOpType.add)
                    elif q > 0:
                        nc.vector.tensor_tensor(
                            mm[:], pbv[:, B - 1, 0:64], pbv[:, B - 1, 1:65],
                            mybir.AluOpType.min)
                    t = s * c + q
                    nc.vector.tensor_tensor_scan(
                        Pb[:, B * 65 + 1:B * 65 + 65], mm[:],
                        Dsk[:, t * kk:(t + 1) * kk],
                        Pb[:, B * 65:B * 65 + 1],
                        mybir.AluOpType.min, mybir.AluOpType.add)
                    if q == c - 1 and s + 1 < S:
                        # next col's prev buffer is this one (cyclic)
                        nc.vector.tensor_tensor(
                            mm[:], pbv[:, B, 0:64], pbv[:, B, 1:65],
                            mybir.AluOpType.min)
            nc.sync.dma_start(out_d[:, :],
                              Pb[:, (2 * c - 1) * 65 + 64:
                                 (2 * c - 1) * 65 + 65])
    return nc


def split_excess_waits(nc):
    """walrus codegen allows ~1 engine-sem + 1 DMA-sem wait per instruction;
    move any excess onto preceding same-engine NoOps."""
    k = 0
    for f in nc.m.functions:
        for blk in f.blocks:
            il = list(blk.instructions)
            out = []
            changed = False
            for inst in il:
                si = getattr(inst, "sync_info", None)
                ow = list(si.on_wait) if si and si.on_wait else []
                if len(ow) > 1:
                    for w in ow[1:]:
                        k += 1
                        nop = mybir.InstNoOp(
                            name=f"wsplit-{k}", engine=inst.engine,
                            bass_nofuse=True,
                            sync_info=mybir.SyncInfo(on_wait=[w],
                                                     on_update=[]))
                        out.append(nop)
                    inst.sync_info = mybir.SyncInfo(
                        on_wait=[ow[0]], on_update=list(si.on_update or []))
                    changed = True
                out.append(inst)
            if changed:
                blk.instructions = out
    return k


_CACHE = {}


def _get_nc(nb, n, m, d):
    key = (nb, n, m, d)
    if key not in _CACHE:
        nc = build_kernel(nb, n, m, d)
        nc.finalize()
        split_excess_waits(nc)
        _CACHE[key] = nc
    return _CACHE[key]


def pack_inputs(x: np.ndarray, y: np.ndarray) -> np.ndarray:
    import ml_dtypes
    B, n, d = x.shape
    m = y.shape[1]
    x = np.ascontiguousarray(x, dtype=np.float32)
    y = np.ascontiguousarray(y, dtype=np.float32)
    allin = np.empty((B, d + 2, n + m), np.float32)
    allin[:, 0:d, 0:n] = x.transpose(0, 2, 1)
    allin[:, d, 0:n] = np.einsum('bnd,bnd->bn', x, x)
    allin[:, d + 1, 0:n] = 1.0
    allin[:, 0:d, n:n + m] = -2.0 * y.transpose(0, 2, 1)
    allin[:, d, n:n + m] = 1.0
    allin[:, d + 1, n:n + m] = np.einsum('bmd,bmd->bm', y, y)
    allin = np.concatenate(
        [allin[:, :, n:n + 128], allin[:, :, 0:n], allin[:, :, n + 128:]],
        axis=2)
    return allin.astype(ml_dtypes.bfloat16)


def make_consts(c=C):
    P = 128
    cons = np.empty((P, 2 + 2 * c), np.float32)
    g0 = (np.arange(P) % G) == 0
    cons[:, 0] = np.where(g0, 0.0, BIG)      # initial halo of buffer 2c-1
    cons[:, 1] = np.where(g0, 0.0, 1.0)      # shuffle-keep mask
    cons[:, 2:] = np.where(g0, BIG, 0.0)[:, None]   # boundary add
    return cons


def prepare_in_maps(x: np.ndarray, y: np.ndarray):
    B = x.shape[0]
    nb = B // NCORES
    allin = pack_inputs(x, y)
    cons = make_consts()
    K = allin.shape[1]
    import ml_dtypes
    fwkk = (G - 1) * 2 * C * (x.shape[1] // G)
    bigarr = np.full((128, fwkk), BIG, ml_dtypes.bfloat16)
    return [{"allin": np.ascontiguousarray(
                allin[cc * nb:(cc + 1) * nb].transpose(1, 0, 2)
                .reshape(K, -1)),
             "consts": cons, "bigd": bigarr}
            for cc in range(NCORES)]


def kernel(x: np.ndarray, y: np.ndarray) -> np.ndarray:
    B, n, d = x.shape
    m = y.shape[1]
    nc = _get_nc(B // NCORES, n, m, d)
    in_maps = prepare_in_maps(x, y)
    res = run_bass_kernel_spmd(nc, in_maps, list(range(NCORES))).results
    return np.concatenate([res[cc]["out"][G - 1::G, 0] for cc in range(NCORES)])
